# revision 1
# baseline (speedup 1.0000x reference)
"""Distributed causal self-attention (RoPE) kernel for 8 TRN2 NeuronCores.

Reference semantics (b=2, s=2048, d=1024, 16 heads, hd=64, fp32):
    q/k/v = x @ W{q,k,v}.T ; q,k = rope(q,k) ; causal softmax(q k^T/sqrt(hd)) @ v ; @ Wo.T

Sharding: core c -> batch (c // 4), head-group (c % 4) [4 heads = 256 dims].
Tensor-parallel column split of Wq/Wk/Wv, row split of Wo; the row-parallel
partial outputs are summed on the host (the unshard for this decomposition).
No device collectives.

Compute dtype: bf16 matmul operands, fp32 PSUM accumulation, fp32 RoPE
tables.  The head-dim basis is permuted per head to [even dims | odd dims]
(dot-product invariant, applied consistently to q and k) so RoPE's
rotate-half partner swap is a clean 32-partition-block swap done by DMA.
Softmax: scores are tiny (|s| < 4) so no max subtraction; exp on ScalarE;
the denominator comes from a ones-column appended to V (row 64 of the
ctx^T matmul accumulator, exact in fp32).
"""

import numpy as np
import ml_dtypes

import concourse.bass as bass
import concourse.mybir as mybir
import concourse.tile as tile
from concourse import bacc
from concourse.bass_utils import run_bass_kernel_spmd

P = 128
B, S, D = 2, 2048, 1024
NH, HD = 16, 64
NCORES = 8
HG = 4                 # heads per core
C = HG * HD            # 256 projected dims per core
THETA = 10000.0
F32 = mybir.dt.float32
BF16 = mybir.dt.bfloat16
BF = ml_dtypes.bfloat16

AX = mybir.AluOpType


def head_perm():
    """Per-head dim permutation: [0,2,...,62, 1,3,...,63]."""
    return np.arange(HD).reshape(HD // 2, 2).T.reshape(-1)


def rope_tables(s=S):
    """cosF/sinF [P, s] fp32 for the T-layout permuted basis.

    Row r (within a 128-row tile covering two heads): freq f = r % 32.
    sinF here is the PRE-SWAP table T with T[q] = S(partner(q)) * sin,
    i.e. +sin on the x1 half (r % 64 < 32), -sin on the x2 half, so that
    after the partner-block swap of t2pre = ps * T the rotate-half term
    lands with the right sign (see build_kernel).
    """
    inv = 1.0 / (THETA ** (np.arange(0, HD, 2, dtype=np.float64) / HD))  # [32]
    pos = np.arange(s, dtype=np.float64)
    r = np.arange(P)
    ang = pos[None, :] * inv[r % 32][:, None]          # [128, s]
    cosf = np.cos(ang).astype(np.float32)
    sign = np.where((r % 64) < 32, 1.0, -1.0)[:, None]
    sinf = (np.sin(ang) * sign).astype(np.float32)
    return cosf, sinf

def build_kernel(s=S, dbg=False, repeat=1):
    """Build the per-core Bass graph (same SPMD graph for all 8 cores).

    Emission order interleaves the second half of the q/k projections with
    the first head-pair's attention so the PE-bound projection work overlaps
    the ScalarE-bound softmax exp.  PSUM budget (8 banks): qk/v projection
    pool 2, scores 2x[128,1024] 4, ctx accumulators 2x[65,512] 2.
    """
    KT = D // P            # k-tiles over the model dim (8)
    CT = C // P            # partition tiles over this core's 256 dims (2)
    TT = s // P            # token tiles (16)
    NEG = -1.0e30

    nc = bacc.Bacc("TRN2", target_bir_lowering=False, debug=False)
    dbg_d = {}
    if dbg:
        for name, shape, dt_ in [
            ("dbg_qT", [P, CT, s], BF16), ("dbg_kT", [P, CT, s], BF16),
            ("dbg_v", [P, TT, HG * (HD + 1)], BF16),
            ("dbg_ctx", [P, CT, s], BF16),
            ("dbg_bc", [64, 512], F32),
            ("dbg_cp", [65, 512], F32),
            ("dbg_rec0", [1, 512], F32),
        ]:
            dbg_d[name] = nc.dram_tensor(name, shape, dt_, kind="ExternalOutput").ap()

    xT_d = nc.dram_tensor("xT", [D, s], BF16, kind="ExternalInput").ap()
    wqT_d = nc.dram_tensor("wqT", [D, C], BF16, kind="ExternalInput").ap()
    wkT_d = nc.dram_tensor("wkT", [D, C], BF16, kind="ExternalInput").ap()
    wvT_d = nc.dram_tensor("wvT", [D, C], BF16, kind="ExternalInput").ap()
    woT_d = nc.dram_tensor("woT", [C, D], BF16, kind="ExternalInput").ap()
    cosf_d = nc.dram_tensor("cosf", [P, s], F32, kind="ExternalInput").ap()
    sinf_d = nc.dram_tensor("sinf", [P, s], F32, kind="ExternalInput").ap()
    out_d = nc.dram_tensor("out", [s, D], F32, kind="ExternalOutput").ap()

    with tile.TileContext(nc) as tc:
      with (
          tc.tile_pool(name="persist", bufs=1) as persist,
          tc.tile_pool(name="small", bufs=3) as small,
      ):
        # ---- persistent SBUF staging ----
        wqT = persist.tile([P, KT, C], BF16, tag="wqT")
        wkT = persist.tile([P, KT, C], BF16, tag="wkT")
        wvT = persist.tile([P, KT, C], BF16, tag="wvT")
        woT = persist.tile([P, CT, D], BF16, tag="woT")
        cosf = persist.tile([P, s], F32, tag="cosf")
        sinf = persist.tile([P, s], F32, tag="sinf")
        qT = persist.tile([P, CT, s], BF16, tag="qT")
        kT = persist.tile([P, CT, s], BF16, tag="kT")
        # v with a ones column per head: [.., h*65+64] == 1.0
        vsb = persist.tile([P, TT, HG * (HD + 1)], BF16, tag="v")
        ctx_pack = persist.tile([P, CT, s], BF16, tag="ctxp")
        ctx_odd = persist.tile([64, CT, s], BF16, tag="ctxo")
        mask = persist.tile([P, P], F32, tag="mask")

        for rep in range(repeat):
            nc.sync.dma_start(wqT[:], wqT_d.rearrange("(a p) c -> p a c", p=P))
            nc.sync.dma_start(wkT[:], wkT_d.rearrange("(a p) c -> p a c", p=P))
            nc.sync.dma_start(wvT[:], wvT_d.rearrange("(a p) c -> p a c", p=P))
            nc.sync.dma_start(woT[:], woT_d.rearrange("(a p) d -> p a d", p=P))
            nc.sync.dma_start(cosf[:], cosf_d)
            nc.sync.dma_start(sinf[:], sinf_d)

            # causal mask tile for scores^T orientation [k-row, q-col]:
            # keep (0) where qcol - krow >= 0, else NEG.
            nc.gpsimd.memset(mask[:], 0.0)
            nc.gpsimd.affine_select(
                out=mask[:], in_=mask[:],
                compare_op=AX.is_ge, fill=NEG,
                base=0, pattern=[[1, P]], channel_multiplier=-1,
            )
            nc.vector.memset(vsb[:], 1.0)

            with tc.tile_pool(name=f"xpool{rep}", bufs=1) as xpool, \
                 tc.tile_pool(name=f"ropet{rep}", bufs=3) as ropet, \
                 tc.tile_pool(name=f"attn{rep}", bufs=18) as attnp, \
                 tc.tile_pool(name=f"qkpsum{rep}", bufs=2, space="PSUM") as qkpsum, \
                 tc.tile_pool(name=f"spsum{rep}", bufs=2, space="PSUM") as spsum, \
                 tc.tile_pool(name=f"cpsum{rep}", bufs=2, space="PSUM") as cpsum:
                xT = xpool.tile([P, KT, s], BF16, tag="xT", name="xT")
                for kt in range(KT):
                    nc.sync.dma_start(
                        xT[:, kt, :],
                        xT_d.rearrange("(a p) s -> p a s", p=P)[:, kt, :])

                # ---- v projection (xT stationary -> natural layout) ----
                for t in range(TT):
                    pv = qkpsum.tile([P, 512], F32, tag="qk", name=f"pv_{t}")
                    for kt in range(KT):
                        nc.tensor.matmul(
                            pv[:, 0:C],
                            lhsT=xT[:, kt, P * t: P * t + P],
                            rhs=wvT[:, kt, :],
                            start=(kt == 0), stop=(kt == KT - 1),
                        )
                    # copy into the ones-augmented v buffer (ScalarE)
                    nc.scalar.copy(
                        vsb[:, t, :].rearrange("p (h e) -> p h e", e=HD + 1)[:, :, 0:HD],
                        pv[:, 0:C].rearrange("p (h e) -> p h e", e=HD),
                    )

                def qk_proj(m):
                    # q/k projections for c-tile m (weights stationary ->
                    # transposed out) + RoPE, in 512-col chunks
                    for wT, outT in [(wqT, qT), (wkT, kT)]:
                        for ck in range(s // 512):
                            fs = 512 * ck
                            ps = qkpsum.tile([P, 512], F32, tag="qk",
                                             name=f"ps_{m}_{ck}")
                            for kt in range(KT):
                                nc.tensor.matmul(
                                    ps[:],
                                    lhsT=wT[:, kt, P * m: P * m + P],
                                    rhs=xT[:, kt, fs: fs + 512],
                                    start=(kt == 0), stop=(kt == KT - 1),
                                )
                            # t2pre[r] = ps[r] * sinF[partner(r)]; partner
                            # swap happens SBUF->SBUF by DMA (DMA cannot read
                            # PSUM; compute engines cannot cross partitions)
                            t2pre = ropet.tile([P, 512], F32, tag="t2pre")
                            nc.vector.tensor_tensor(
                                t2pre[:], ps[:], sinf[:, fs: fs + 512],
                                op=AX.mult)
                            t2 = ropet.tile([P, 512], F32, tag="t2")
                            for blk in range(4):
                                src = (blk ^ 1) * 32
                                nc.sync.dma_start(
                                    t2[32 * blk: 32 * blk + 32, :],
                                    t2pre[src: src + 32, :])
                            t1 = ropet.tile([P, 512], F32, tag="t1")
                            nc.vector.tensor_tensor(
                                t1[:], ps[:], cosf[:, fs: fs + 512],
                                op=AX.mult)
                            nc.vector.tensor_tensor(
                                outT[:, m, fs: fs + 512], t1[:], t2[:],
                                op=AX.add)

                def attention(hpair):
                    ch = hpair
                    for w in range(s // 512):     # 512-wide q windows
                        ws = 512 * w
                        jmax = (ws + 512) // 128
                        cps = {h2: cpsum.tile([65, 512], F32, tag="c",
                                              name=f"cp_{hpair}_{w}_{h2}")
                               for h2 in range(2)}
                        for j in range(jmax):
                            start = max(ws, 128 * j)
                            d = start - ws
                            # scores for BOTH heads into one [A|B] psum tile;
                            # adjacent matmuls run concurrently in the two
                            # PE row-halves (K=64 each)
                            sc = spsum.tile([P, 1024], F32, tag="s",
                                            name=f"sc_{hpair}_{w}_{j}")
                            for h2 in range(2):
                                rh = 64 * h2
                                nc.tensor.matmul(
                                    sc[:, 512 * h2 + d: 512 * h2 + 512],
                                    lhsT=kT[rh: rh + 64, ch,
                                            128 * j: 128 * j + 128],
                                    rhs=qT[rh: rh + 64, ch, start: ws + 512],
                                    start=True, stop=True,
                                )
                            if 128 * j >= ws:
                                # this k-tile contains the diagonal block
                                for h2 in range(2):
                                    scv = sc[:, 512 * h2 + d: 512 * h2 + d + P]
                                    nc.vector.tensor_tensor(
                                        scv, scv, mask[:], op=AX.add)
                            at = attnp.tile([P, 1024], BF16, tag="attn",
                                            name=f"at_{hpair}_{w}_{j}")
                            if d > 0:
                                nc.gpsimd.memset(at[:, 0: d], 0.0)
                                nc.gpsimd.memset(at[:, 512: 512 + d], 0.0)
                            # ONE wide exp covering both heads' valid cols
                            nc.scalar.activation(
                                at[:].rearrange(
                                    "p (b n) -> p b n", b=2)[:, :, d: 512],
                                sc[:].rearrange(
                                    "p (b n) -> p b n", b=2)[:, :, d: 512],
                                mybir.ActivationFunctionType.Exp,
                                bias=0.0, scale=0.125,
                            )
                            for h2 in range(2):
                                h = 2 * hpair + h2
                                nc.tensor.matmul(
                                    cps[h2][:, d:512],
                                    lhsT=vsb[:, j,
                                             (HD + 1) * h: (HD + 1) * h + HD + 1],
                                    rhs=at[:, 512 * h2 + d: 512 * h2 + 512],
                                    start=(j == 0), stop=(j == jmax - 1),
                                )
                        for h2 in range(2):
                            cp = cps[h2]
                            rec = small.tile([65, 512], F32, tag="rec")
                            # the custom DVE op mishandles partition-offset
                            # PSUM APs on HW: reciprocal the whole base-0
                            # slice (same per-lane cycles); only row 64 (the
                            # denominator row) is consumed.
                            nc.vector.reciprocal_approx_fast(
                                out=rec[0:65, :], in_=cp[0:65, :])
                            # HW partition_broadcast only reads partition 0:
                            # hop the recip row down via a tiny DMA first.
                            rec0 = small.tile([1, 512], F32, tag="rec0")
                            nc.gpsimd.dma_start(rec0[:], rec[64:65, :])
                            bcast = small.tile([64, 512], F32, tag="bc")
                            nc.gpsimd.partition_broadcast(
                                bcast[:], rec0[0:1, :])
                            if dbg and hpair == 0 and w == 0 and h2 == 0:
                                nc.sync.dma_start(dbg_d["dbg_bc"], bcast[:])
                                cpd = small.tile([65, 512], F32, tag="cpd",
                                                 name="cpdbg")
                                nc.scalar.copy(cpd[:], cp[:])
                                nc.sync.dma_start(dbg_d["dbg_cp"], cpd[:])
                                nc.sync.dma_start(dbg_d["dbg_rec0"], rec0[:])
                            dst = ctx_pack if h2 == 0 else ctx_odd
                            nc.vector.tensor_tensor(
                                dst[0:64, ch, ws: ws + 512],
                                cp[0:64, :], bcast[:], op=AX.mult)

                qk_proj(0)
                attention(0)
                nc.gpsimd.dma_start(ctx_pack[64:128, 0, :], ctx_odd[0:64, 0, :])
                qk_proj(1)
                attention(1)
                nc.gpsimd.dma_start(ctx_pack[64:128, 1, :], ctx_odd[0:64, 1, :])

            if dbg:
                nc.sync.dma_start(dbg_d["dbg_qT"], qT[:])
                nc.sync.dma_start(dbg_d["dbg_kT"], kT[:])
                nc.sync.dma_start(dbg_d["dbg_v"], vsb[:])
                nc.sync.dma_start(dbg_d["dbg_ctx"], ctx_pack[:])

            # ---- output projection ----
            with tc.tile_pool(name=f"opsum{rep}", bufs=4, space="PSUM") as opsum, \
                 tc.tile_pool(name=f"ostage{rep}", bufs=3) as ostage:
                for t in range(TT):
                    ot = ostage.tile([P, D], F32, tag="ot", name=f"ot_{t}")
                    for nchunk in range(2):
                        po = opsum.tile([P, 512], F32, tag="o",
                                        name=f"po_{t}_{nchunk}")
                        for ct in range(CT):
                            nc.tensor.matmul(
                                po[:],
                                lhsT=ctx_pack[:, ct, P * t: P * t + P],
                                rhs=woT[:, ct, 512 * nchunk: 512 * nchunk + 512],
                                start=(ct == 0), stop=(ct == CT - 1),
                            )
                        if (t + nchunk) % 2 == 0:
                            nc.scalar.copy(
                                ot[:, 512 * nchunk: 512 * nchunk + 512], po[:])
                        else:
                            nc.vector.tensor_copy(
                                ot[:, 512 * nchunk: 512 * nchunk + 512], po[:])
                    nc.sync.dma_start(out_d[P * t: P * t + P, :], ot[:])

    nc.compile()
    return nc

def make_in_maps(x, Wq, Wk, Wv, Wo, s=S):
    """Host-side shard prep: per-core input dict."""
    perm = head_perm()
    cosf, sinf = rope_tables(s)
    in_maps = []
    for c in range(NCORES):
        bi, hg = c // HG, c % HG
        heads = np.arange(HG * hg, HG * hg + HG)
        pcols = np.concatenate([h * HD + perm for h in heads])   # permuted q/k cols
        vcols = np.concatenate([h * HD + np.arange(HD) for h in heads])
        in_maps.append({
            "xT": np.ascontiguousarray(x[bi].T).astype(BF),
            "wqT": np.ascontiguousarray(Wq[pcols, :].T).astype(BF),
            "wkT": np.ascontiguousarray(Wk[pcols, :].T).astype(BF),
            "wvT": np.ascontiguousarray(Wv[vcols, :].T).astype(BF),
            "woT": np.ascontiguousarray(Wo[:, vcols].T).astype(BF),
            "cosf": cosf,
            "sinf": sinf,
        })
    return in_maps


_CACHE = {}


def _compiled(s=S):
    if s not in _CACHE:
        _CACHE[s] = build_kernel(s)
    return _CACHE[s]


def kernel(x, Wq, Wk, Wv, Wo, trace=False):
    x = np.asarray(x, dtype=np.float32)
    in_maps = make_in_maps(x, np.asarray(Wq), np.asarray(Wk),
                           np.asarray(Wv), np.asarray(Wo))
    nc = _compiled()
    res = run_bass_kernel_spmd(nc, in_maps, core_ids=list(range(NCORES)),
                               trace=trace)
    out = np.zeros((B, S, D), dtype=np.float32)
    for c in range(NCORES):
        out[c // HG] += res.results[c]["out"]
    if trace:
        return out, res
    return out



# revision 9
# speedup vs baseline: 2.9543x; 2.9543x over previous
"""Distributed causal self-attention (RoPE) kernel for 8 TRN2 NeuronCores.

Reference semantics (b=2, s=2048, d=1024, 16 heads, hd=64, fp32):
    q/k/v = x @ W{q,k,v}.T ; q,k = rope(q,k) ; causal softmax(q k^T/sqrt(hd)) @ v ; @ Wo.T

Sharding: core c -> batch (c // 4), head-group (c % 4) [4 heads = 256 dims].
Tensor-parallel column split of Wq/Wk/Wv, row split of Wo; the row-parallel
partial outputs are summed on the host (the unshard for this decomposition).
No device collectives.

HW findings this kernel is shaped around (measured, repeat-delta):
  - A DMA queue processes its descriptors serially at ~22 GB/s with a
    per-partition-line overhead, so (a) every HBM tensor is host-prepacked
    to its exact SBUF layout => one DMA with fat contiguous lines, and
    (b) big transfers are spread across the three issue paths (SP-HWDGE,
    Act-HWDGE, Pool-SWDGE).
  - The softmax normalize chain was >half of attention time; the recip
    row's hop to partition 0 rides the low-latency HWDGE queues and the
    chain overlaps the next window's compute.
  - PE stalled behind ScalarE's exp each k-block; emission software-
    pipelines scores(j+1) ahead of attnV(j), and a wide causal mask
    (add, exp(NEG)->0) replaces the per-block edge memsets.
  - Output is staged bf16 and written per 512-token window, alternating
    SP/Act queues, so the store DMA overlaps attention compute.

Compute dtype: bf16 matmul operands, fp32 PSUM accumulation, fp32 RoPE
tables.  The head-dim basis is permuted per head to [even dims | odd dims]
(dot-product invariant, applied consistently to q and k) so RoPE's
rotate-half partner swap is a clean 32-partition-block swap done by DMA.
Softmax: scores are tiny (|s| < 4) so no max subtraction; exp on ScalarE.
"""

import numpy as np
import ml_dtypes

import concourse.bass as bass
import concourse.mybir as mybir
import concourse.tile as tile
from concourse import bacc
from concourse.bass_utils import run_bass_kernel_spmd

P = 128
B, S, D = 2, 2048, 1024
NH, HD = 16, 64
NCORES = 8
HG = 4                 # heads per core
C = HG * HD            # 256 projected dims per core
THETA = 10000.0
F32 = mybir.dt.float32
BF16 = mybir.dt.bfloat16
BF = ml_dtypes.bfloat16

AX = mybir.AluOpType

KT = D // P            # k-tiles over the model dim (8)
CT = C // P            # partition tiles over this core's 256 dims (2)


def head_perm():
    """Per-head dim permutation: [0,2,...,62, 1,3,...,63]."""
    return np.arange(HD).reshape(HD // 2, 2).T.reshape(-1)


def rope_tables(s=S):
    """cosF/sinF [P, s] fp32 for the T-layout permuted basis.

    Row r (within a 128-row tile covering two heads): freq f = r % 32.
    sinF here is the PRE-SWAP table T with T[q] = S(partner(q)) * sin,
    i.e. +sin on the x1 half (r % 64 < 32), -sin on the x2 half, so that
    after the partner-block swap of t2pre = ps * T the rotate-half term
    lands with the right sign (see build_kernel).
    """
    inv = 1.0 / (THETA ** (np.arange(0, HD, 2, dtype=np.float64) / HD))  # [32]
    pos = np.arange(s, dtype=np.float64)
    r = np.arange(P)
    ang = pos[None, :] * inv[r % 32][:, None]          # [128, s]
    cosf = np.cos(ang).astype(np.float32)
    sign = np.where((r % 64) < 32, 1.0, -1.0)[:, None]
    sinf = (np.sin(ang) * sign).astype(np.float32)
    return cosf, sinf


def _pack_rows(a, kt):
    """[kt*128, n] -> [128, kt*n] so each SBUF partition line is contiguous."""
    n = a.shape[1]
    return np.ascontiguousarray(
        a.reshape(kt, P, n).transpose(1, 0, 2).reshape(P, kt * n))


def build_kernel(s=S, repeat=1, reps=None):
    """Per-core Bass graph (same SPMD graph on all 8 cores)."""
    TT = s // P            # token tiles (16)
    NW = s // 512          # 512-wide q windows (4)
    NEG = -1.0e30
    R = {"load": 1, "proj": 1, "attn": 1, "out": 1}
    if reps:
        R.update(reps)

    nc = bacc.Bacc("TRN2", target_bir_lowering=False, debug=False)

    xTp_d = nc.dram_tensor("xTp", [P, KT * s], BF16, kind="ExternalInput").ap()
    wqT_d = nc.dram_tensor("wqTp", [P, KT * C], BF16, kind="ExternalInput").ap()
    wkT_d = nc.dram_tensor("wkTp", [P, KT * C], BF16, kind="ExternalInput").ap()
    wvT_d = nc.dram_tensor("wvTp", [P, KT * C], BF16, kind="ExternalInput").ap()
    woT_d = nc.dram_tensor("woTp", [P, CT * D], BF16, kind="ExternalInput").ap()
    cosf_d = nc.dram_tensor("cosf", [P, s], F32, kind="ExternalInput").ap()
    sinf_d = nc.dram_tensor("sinf", [P, s], F32, kind="ExternalInput").ap()
    out_d = nc.dram_tensor("out", [s, D], BF16, kind="ExternalOutput").ap()

    with tile.TileContext(nc) as tc:
      with (
          tc.tile_pool(name="persist", bufs=1) as persist,
          tc.tile_pool(name="small", bufs=6) as small,
      ):
        # ---- persistent SBUF staging ----
        wqT = persist.tile([P, KT, C], BF16, tag="wqT")
        wkT = persist.tile([P, KT, C], BF16, tag="wkT")
        wvT = persist.tile([P, KT, C], BF16, tag="wvT")
        woT = persist.tile([P, CT, D], BF16, tag="woT")
        cosf = persist.tile([P, s], F32, tag="cosf")
        sinf = persist.tile([P, s], F32, tag="sinf")
        qT = persist.tile([P, CT, s], BF16, tag="qT")
        kT = persist.tile([P, CT, s], BF16, tag="kT")
        # v with a ones column per head: [.., h*65+64] == 1.0 (den at cp
        # row 64; PSUM APs must start at partition 0, so ones-first is not
        # compilable -- the recip row hops to partition 0 by HWDGE DMA).
        vsb = persist.tile([P, TT, HG * (HD + 1)], BF16, tag="v")
        ctx_pack = persist.tile([P, CT, s], BF16, tag="ctxp")
        ctx_odd = persist.tile([64, CT, s], BF16, tag="ctxo")
        # wide causal mask for [k-row r, q-col] diagonal blocks:
        # slice [384-d : 896-d] gives col c masked (NEG) iff c - d < r.
        wmask = persist.tile([P, 896], F32, tag="wmask")

        for rep in range(repeat):
            for _li in range(R["load"]):
                # spread the big loads over the three DMA issue paths
                nc.sync.dma_start(cosf[:], cosf_d)
                nc.scalar.dma_start(sinf[:], sinf_d)
                nc.gpsimd.dma_start(
                    wvT[:], wvT_d.rearrange("p (a c) -> p a c", c=C))
                nc.gpsimd.dma_start(
                    wqT[:], wqT_d.rearrange("p (a c) -> p a c", c=C))
                nc.gpsimd.dma_start(
                    wkT[:], wkT_d.rearrange("p (a c) -> p a c", c=C))
                nc.gpsimd.dma_start(
                    woT[:], woT_d.rearrange("p (a d) -> p a d", d=D))

            nc.gpsimd.memset(wmask[:], 0.0)
            nc.gpsimd.affine_select(
                out=wmask[:], in_=wmask[:],
                compare_op=AX.is_ge, fill=NEG,
                base=-384, pattern=[[1, 896]], channel_multiplier=-1,
            )
            nc.vector.memset(vsb[:], 1.0)

            with tc.tile_pool(name=f"xpool{rep}", bufs=1) as xpool, \
                 tc.tile_pool(name=f"ropet{rep}", bufs=3) as ropet, \
                 tc.tile_pool(name=f"attn{rep}", bufs=8) as attnp, \
                 tc.tile_pool(name=f"ostage{rep}", bufs=3) as ostage:
                xT = xpool.tile([P, KT, s], BF16, tag="xT", name="xT")
                for _li in range(R["load"]):
                    for h in range(4):      # 2 kt per DMA, alternate queues
                        q_ = nc.sync if h % 2 == 0 else nc.scalar
                        q_.dma_start(
                            xT[:, 2 * h: 2 * h + 2, :],
                            xTp_d.rearrange("p (a t) -> p a t", t=s)
                            [:, 2 * h: 2 * h + 2, :])

                for _pi in range(R["proj"]):
                  with tc.tile_pool(name=f"qkpsum{rep}_{_pi}", bufs=2,
                                    space="PSUM") as qkpsum:
                    # ---- v projection (xT stationary -> natural layout) ----
                    for t in range(TT):
                        pv = qkpsum.tile([P, 512], F32, tag="qk",
                                         name=f"pv_{t}")
                        for kt in range(KT):
                            nc.tensor.matmul(
                                pv[:, 0:C],
                                lhsT=xT[:, kt, P * t: P * t + P],
                                rhs=wvT[:, kt, :],
                                start=(kt == 0), stop=(kt == KT - 1),
                            )
                        # ones-last augmented v buffer (ScalarE)
                        nc.scalar.copy(
                            vsb[:, t, :].rearrange(
                                "p (h e) -> p h e", e=HD + 1)[:, :, 0:HD],
                            pv[:, 0:C].rearrange("p (h e) -> p h e", e=HD),
                        )

                    # ---- q/k projections (weights stationary -> qT/kT) ----
                    for m in range(CT):
                        for wT, outT in [(wqT, qT), (wkT, kT)]:
                            for ck in range(s // 512):
                                fs = 512 * ck
                                ps = qkpsum.tile([P, 512], F32, tag="qk",
                                                 name=f"ps_{m}_{ck}")
                                for kt in range(KT):
                                    nc.tensor.matmul(
                                        ps[:],
                                        lhsT=wT[:, kt, P * m: P * m + P],
                                        rhs=xT[:, kt, fs: fs + 512],
                                        start=(kt == 0), stop=(kt == KT - 1),
                                    )
                                # t2pre[r] = ps[r] * sinF[partner(r)]; the
                                # partner swap is SBUF->SBUF by DMA (DMA can't
                                # read PSUM; compute can't cross partitions)
                                t2pre = ropet.tile([P, 512], F32, tag="t2pre")
                                nc.vector.tensor_tensor(
                                    t2pre[:], ps[:], sinf[:, fs: fs + 512],
                                    op=AX.mult)
                                t2 = ropet.tile([P, 512], F32, tag="t2")
                                for blk in range(4):
                                    src = (blk ^ 1) * 32
                                    (nc.sync if ck % 2 == 0
                                     else nc.scalar).dma_start(
                                        t2[32 * blk: 32 * blk + 32, :],
                                        t2pre[src: src + 32, :])
                                t1 = ropet.tile([P, 512], F32, tag="t1")
                                nc.vector.tensor_tensor(
                                    t1[:], ps[:], cosf[:, fs: fs + 512],
                                    op=AX.mult)
                                nc.vector.tensor_tensor(
                                    outT[:, m, fs: fs + 512], t1[:], t2[:],
                                    op=AX.add)

                with tc.tile_pool(name=f"spsum{rep}", bufs=2,
                                  space="PSUM") as spsum, \
                     tc.tile_pool(name=f"cpsum{rep}", bufs=2,
                                  space="PSUM") as cpsum, \
                     tc.tile_pool(name=f"opsum{rep}", bufs=2,
                                  space="PSUM") as opsum:

                    def out_proj(w):
                        for t in range(4 * w, 4 * w + 4):
                            ot = ostage.tile([P, D], BF16, tag="ot",
                                             name=f"ot_{t}")
                            for nk in range(2):
                                po = opsum.tile([P, 512], F32, tag="o",
                                                name=f"po_{t}_{nk}")
                                for ct in range(CT):
                                    nc.tensor.matmul(
                                        po[:],
                                        lhsT=ctx_pack[:, ct, P * t: P * t + P],
                                        rhs=woT[:, ct,
                                                512 * nk: 512 * nk + 512],
                                        start=(ct == 0), stop=(ct == CT - 1),
                                    )
                                if (t + nk) % 2 == 0:
                                    nc.scalar.copy(
                                        ot[:, 512 * nk: 512 * nk + 512], po[:])
                                else:
                                    nc.vector.tensor_copy(
                                        ot[:, 512 * nk: 512 * nk + 512], po[:])
                            (nc.sync if t % 2 == 0 else nc.scalar).dma_start(
                                out_d[P * t: P * t + P, :], ot[:])

                    def window(w, ch):
                        ws = 512 * w
                        jmax = (ws + 512) // 128
                        cps = {h2: cpsum.tile([65, 512], F32, tag="c",
                                              name=f"cp_{ch}_{w}_{h2}")
                               for h2 in range(2)}
                        pend = None
                        for j in range(jmax):
                            sc = spsum.tile([P, 1024], F32, tag="s",
                                            name=f"sc_{ch}_{w}_{j}")
                            for h2 in range(2):
                                rh = 64 * h2
                                nc.tensor.matmul(
                                    sc[:, 512 * h2: 512 * h2 + 512],
                                    lhsT=kT[rh: rh + 64, ch,
                                            128 * j: 128 * j + 128],
                                    rhs=qT[rh: rh + 64, ch, ws: ws + 512],
                                    start=True, stop=True,
                                )
                            if 128 * j >= ws:        # diagonal block
                                d = 128 * j - ws
                                for h2 in range(2):
                                    scv = sc[:, 512 * h2: 512 * h2 + 512]
                                    nc.vector.tensor_tensor(
                                        scv, scv, wmask[:, 384 - d: 896 - d],
                                        op=AX.add)
                            at = attnp.tile([P, 1024], BF16, tag="attn",
                                            name=f"at_{ch}_{w}_{j}")
                            nc.scalar.activation(
                                at[:], sc[:],
                                mybir.ActivationFunctionType.Exp,
                                bias=0.0, scale=0.125,
                            )
                            if pend is not None:
                                pj, pat = pend
                                for h2 in range(2):
                                    nc.tensor.matmul(
                                        cps[h2][:],
                                        lhsT=vsb[:, pj, (HD + 1) * (2 * ch + h2):
                                                 (HD + 1) * (2 * ch + h2) + HD + 1],
                                        rhs=pat[:, 512 * h2: 512 * h2 + 512],
                                        start=(pj == 0), stop=False,
                                    )
                            pend = (j, at)
                        pj, pat = pend
                        for h2 in range(2):
                            nc.tensor.matmul(
                                cps[h2][:],
                                lhsT=vsb[:, pj, (HD + 1) * (2 * ch + h2):
                                         (HD + 1) * (2 * ch + h2) + HD + 1],
                                rhs=pat[:, 512 * h2: 512 * h2 + 512],
                                start=(pj == 0), stop=True,
                            )
                        # normalize: den is cp row 64; recip the base-0
                        # slice (PSUM APs must start at partition 0), hop the
                        # recip row to partition 0 by HWDGE DMA for broadcast
                        for h2 in range(2):
                            cp = cps[h2]
                            rec = small.tile([65, 512], F32, tag="rec")
                            nc.vector.reciprocal_approx_fast(
                                out=rec[0:65, :], in_=cp[0:65, :])
                            rec0 = small.tile([1, 512], F32, tag="rec0")
                            (nc.sync if h2 == 0 else nc.scalar).dma_start(
                                rec0[:], rec[64:65, :])
                            bcast = small.tile([64, 512], F32, tag="bc")
                            nc.gpsimd.partition_broadcast(
                                bcast[:], rec0[0:1, :])
                            dst = ctx_pack if h2 == 0 else ctx_odd
                            nc.vector.tensor_tensor(
                                dst[0:64, ch, ws: ws + 512],
                                cp[0:64, :], bcast[:], op=AX.mult)
                        nc.gpsimd.dma_start(
                            ctx_pack[64:128, ch, ws: ws + 512],
                            ctx_odd[0:64, ch, ws: ws + 512])

                    for _ai in range(R["attn"]):
                        for w in range(NW):
                            window(w, 0)
                            if w > 0:
                                for _oi in range(R["out"]):
                                    out_proj(w - 1)
                            window(w, 1)
                        for _oi in range(R["out"]):
                            out_proj(NW - 1)

    nc.compile()
    return nc


def make_in_maps(x, Wq, Wk, Wv, Wo, s=S):
    """Host-side shard prep: per-core input dict, prepacked to SBUF layout."""
    perm = head_perm()
    cosf, sinf = rope_tables(s)
    in_maps = []
    for c in range(NCORES):
        bi, hg = c // HG, c % HG
        heads = np.arange(HG * hg, HG * hg + HG)
        pcols = np.concatenate([h * HD + perm for h in heads])   # permuted q/k
        vcols = np.concatenate([h * HD + np.arange(HD) for h in heads])
        in_maps.append({
            "xTp": _pack_rows(np.ascontiguousarray(x[bi].T).astype(BF), KT),
            "wqTp": _pack_rows(
                np.ascontiguousarray(Wq[pcols, :].T).astype(BF), KT),
            "wkTp": _pack_rows(
                np.ascontiguousarray(Wk[pcols, :].T).astype(BF), KT),
            "wvTp": _pack_rows(
                np.ascontiguousarray(Wv[vcols, :].T).astype(BF), KT),
            "woTp": _pack_rows(
                np.ascontiguousarray(Wo[:, vcols].T).astype(BF), CT),
            "cosf": cosf,
            "sinf": sinf,
        })
    return in_maps


_CACHE = {}


def _compiled(s=S):
    if s not in _CACHE:
        _CACHE[s] = build_kernel(s)
    return _CACHE[s]


def kernel(x, Wq, Wk, Wv, Wo, trace=False):
    x = np.asarray(x, dtype=np.float32)
    in_maps = make_in_maps(x, np.asarray(Wq), np.asarray(Wk),
                           np.asarray(Wv), np.asarray(Wo))
    nc = _compiled()
    res = run_bass_kernel_spmd(nc, in_maps, core_ids=list(range(NCORES)),
                               trace=trace)
    out = np.zeros((B, S, D), dtype=np.float32)
    for c in range(NCORES):
        out[c // HG] += np.asarray(res.results[c]["out"]).astype(np.float32)
    if trace:
        return out, res
    return out
